# revision 4
# baseline (speedup 1.0000x reference)
"""Trainium2 Bass kernel for the Gaussian-alignment attention (Aligner) module.

Computes, for B=16, C=512, TX=1024, TY=2048:
    centers[b,t] = cumsum(x_lengths)[b,t] - 0.5*x_lengths[b,t]
    logits[b,t,y] = -(centers[b,t] - (y + y_offset[b]))^2 / 10 - 1e9*(1-mask[b,t])
    out[b,c,y] = sum_t x_latents[b,c,t] * softmax_t(logits)[b,t,y]

Strategy: data-parallel over batch across 8 NeuronCores (2 batches/core).
The alignment matrix is a narrow diagonal band (Gaussian with sigma ~2.2 in
position units), so exp() weights and the contraction are only evaluated on
data-dependent 128-wide t-blocks per 128-wide y-tile.  Per y-tile:
    E[t,y] = exp((-(c[t]-y)^2 + d2min[y]) / 10)   (ACT Square w/ per-partition
                                                   bias -> optional DVE shift
                                                   subtract -> ACT Exp)
    num    = E^T(y,t) . [x*mask]^T(t,c)           (PE matmul, t contracted)
    den    = E^T(y,t) . mask(t)                   (PE matmul, N=2 padded)
    out[y,c] = num * (1/den)                      (DVE reciprocal + t_scalar)
The d2min[y] shift is the exact softmax max-subtraction of the reference; it
is only materialized for t-blocks where some output frame is far from every
center (cumsum tails).  Masked tokens get their center pushed to +1e9 so
their weights are exactly 0.  The instruction stream is shared by all 8
cores (SPMD), so windows are unioned over all 16 batches.
"""

import os
import sys

sys.path.insert(0, "/opt/trn_rl_repo")

import numpy as np

B, C, TX, TY = 16, 512, 1024, 2048
GAMMA = 10.0
N_CORES = 8
NB = B // N_CORES          # batches per core
NPO = TX // 128            # t-blocks
NYT = TY // 128            # y-tiles
PAD_D2 = 1050.0            # keep terms until exp() underflows past subnormals
SHIFT_NEEDED_D2 = 600.0    # frames with d2min above this get the exact shift
FAR_CENTER = 1.0e9         # masked-token center placement


def _np_reference(x_latents, x_lengths, x_mask, y_len, y_offset):
    """Exact-semantics numpy fallback (used only for pathological inputs)."""
    out = np.empty((B, C, TY), dtype=np.float32)
    for b in range(B):
        ends = np.cumsum(x_lengths[b], dtype=np.float32)
        cen = (ends - np.float32(0.5) * x_lengths[b]).astype(np.float32)
        pos = (np.arange(TY, dtype=np.int64) + int(y_offset[b])).astype(np.float32)
        dist = cen[:, None] - pos[None, :]
        logits = -(dist * dist) / np.float32(GAMMA)
        logits = logits - np.float32(1e9) * (np.float32(1.0) - x_mask[b, 0][:, None])
        m = logits.max(axis=0, keepdims=True)
        w = np.exp(logits - m)
        w = w / w.sum(axis=0, keepdims=True)
        out[b] = x_latents[b].astype(np.float32) @ w
    return out


def _plan(centers_dev, mask):
    """Per-y-tile t-block windows, per-t-block y-strips and shift needs,
    unioned over all batches (the SPMD program is shared by all cores).

    centers_dev: [B, TX] float64, masked entries already at FAR_CENTER.
    Returns (windows, strips, sneed, needy) or None (-> numpy fallback).
      windows[yt] = (po_lo, po_hi) inclusive
      strips[po]  = (ys, ye) or None
      sneed       = [B, TY] float32 shift rows (thresholded d2min, the
                    reference's softmax max-subtraction where it matters)
      needy       = set of po whose strip applies the shift subtract
    """
    pos = np.arange(TY, dtype=np.float64)
    d2min = np.empty((B, TY), dtype=np.float32)
    for b in range(B):
        cm = centers_dev[b][centers_dev[b] < FAR_CENTER / 2]
        if cm.size == 0:
            return None
        cs = np.sort(cm)
        idx = np.searchsorted(cs, pos)
        lo = np.clip(idx - 1, 0, cs.size - 1)
        hi = np.clip(idx, 0, cs.size - 1)
        dmin = np.minimum(np.abs(cs[lo] - pos), np.abs(cs[hi] - pos))
        d2min[b] = (dmin * dmin).astype(np.float32)

    windows = []
    for yt in range(NYT):
        y0, y1 = 128 * yt, 128 * yt + 127
        r = float(np.sqrt(d2min[:, y0:y1 + 1].max() + PAD_D2))
        plo, phi = NPO, -1
        for b in range(B):
            t = np.nonzero((centers_dev[b] >= y0 - r) &
                           (centers_dev[b] <= y1 + r))[0]
            if t.size:
                plo = min(plo, int(t[0]) // 128)
                phi = max(phi, int(t[-1]) // 128)
        if phi < 0:
            return None
        windows.append((plo, phi))

    strips = [None] * NPO
    for po in range(NPO):
        yts = [yt for yt in range(NYT) if windows[yt][0] <= po <= windows[yt][1]]
        if yts:
            strips[po] = (128 * min(yts), 128 * max(yts) + 128)

    # The shift is the reference's softmax max-subtraction; it must be applied
    # to either all or none of a frame's terms, so threshold it to zero except
    # for deep-tail frames and close "needy" over whole y-tile windows.
    sneed = np.where(d2min > SHIFT_NEEDED_D2, d2min, 0.0).astype(np.float32)
    needy = set()
    for yt in range(NYT):
        y0 = 128 * yt
        if sneed[:, y0:y0 + 128].max() > 0:
            needy.update(range(windows[yt][0], windows[yt][1] + 1))
    return windows, strips, sneed, needy


def _build_program(windows, strips, needy, use_fp32r):
    import concourse.mybir as mybir
    import concourse.tile as tile
    from concourse import bacc

    F32 = mybir.dt.float32
    DT = mybir.dt.float32r if use_fp32r else F32
    ACTF = mybir.ActivationFunctionType

    nc = bacc.Bacc("TRN2", target_bir_lowering=False, debug=False,
                   num_devices=N_CORES)
    xt = nc.dram_tensor("xt", [NB, TX, C + 2], DT, kind="ExternalInput")
    cen = nc.dram_tensor("cen", [NB, 128, NPO], F32, kind="ExternalInput")
    ar = nc.dram_tensor("ar", [128, TY], F32, kind="ExternalInput")
    sh = {po: nc.dram_tensor(f"sh{po}", [NB, 128, strips[po][1] - strips[po][0]],
                             F32, kind="ExternalInput")
          for po in sorted(needy)}
    out = nc.dram_tensor("out", [NB, TY, C], F32, kind="ExternalOutput")

    with tile.TileContext(nc) as tc:
        with (
            tc.tile_pool(name="const", bufs=1) as constp,
            tc.tile_pool(name="xtp", bufs=2) as xtp,
            tc.tile_pool(name="cenp", bufs=2) as cenp,
            tc.tile_pool(name="sqp", bufs=2) as sqp,
            tc.tile_pool(name="shp", bufs=2) as shp,
            tc.tile_pool(name="ep", bufs=2) as epool,
            tc.tile_pool(name="pnum", bufs=1, space="PSUM") as pnum_pool,
            tc.tile_pool(name="pden", bufs=1, space="PSUM") as pden_pool,
            tc.tile_pool(name="wrk", bufs=4) as wrk,
            tc.tile_pool(name="outp", bufs=3) as outp,
        ):
            ar_sb = constp.tile([128, TY], F32)
            nc.sync.dma_start(ar_sb[:], ar[:])

            for slot in range(NB):
                xt_sb = xtp.tile([128, NPO, C + 2], DT, tag="xt")
                nc.sync.dma_start(
                    xt_sb[:], xt[slot].rearrange("(po pi) c -> pi po c", pi=128))
                cen_sb = cenp.tile([128, NPO], F32, tag="cen")
                nc.sync.dma_start(cen_sb[:], cen[slot])

                e_strips = {}
                for po in range(NPO):
                    if strips[po] is None:
                        continue
                    ys, ye = strips[po]
                    sq = sqp.tile([128, ye - ys], F32, tag=f"sq{po}")
                    nc.scalar.activation(sq[:], ar_sb[:, ys:ye], ACTF.Square,
                                         bias=cen_sb[:, po:po + 1], scale=-1.0)
                    if po in needy:
                        sh_sb = shp.tile([128, ye - ys], F32, tag=f"sh{po}")
                        nc.sync.dma_start(sh_sb[:], sh[po][slot])
                        nc.vector.tensor_sub(sq[:], sq[:], sh_sb[:])
                    ep = epool.tile([128, ye - ys], DT, tag=f"e{po}")
                    nc.scalar.activation(ep[:], sq[:], ACTF.Exp,
                                         scale=-1.0 / GAMMA)
                    e_strips[po] = ep

                for yt in range(NYT):
                    y0 = 128 * yt
                    plo, phi = windows[yt]
                    pnum = pnum_pool.tile([128, C], mybir.dt.float32,
                                          tag=f"pn{yt % 3}")
                    pden = pden_pool.tile([128, 2], mybir.dt.float32,
                                          tag=f"pd{yt % 4}")
                    for j, po in enumerate(range(plo, phi + 1)):
                        ys, _ = strips[po]
                        w_ap = e_strips[po][:, y0 - ys:y0 - ys + 128]
                        nc.tensor.matmul(pnum[:], w_ap, xt_sb[:, po, 0:C],
                                         start=(j == 0), stop=(po == phi))
                        nc.tensor.matmul(pden[:], w_ap, xt_sb[:, po, C:C + 2],
                                         start=(j == 0), stop=(po == phi))
                    recip = wrk.tile([128, 1], F32, tag="recip")
                    nc.vector.reciprocal(recip[:], pden[:, 0:1])
                    ob = outp.tile([128, C], F32, tag=f"ob{yt % 3}")
                    nc.vector.tensor_scalar_mul(ob[:], pnum[:], recip[:])
                    nc.sync.dma_start(out[slot, y0:y0 + 128, :], ob[:])

    nc.finalize()
    return nc


def kernel(x_latents, x_lengths, x_mask, y_len, y_offset):
    x_latents = np.ascontiguousarray(np.asarray(x_latents, dtype=np.float32))
    x_lengths = np.ascontiguousarray(np.asarray(x_lengths, dtype=np.float32))
    x_mask = np.ascontiguousarray(np.asarray(x_mask, dtype=np.float32))
    y_len = np.asarray(y_len)
    y_offset = np.asarray(y_offset)
    assert x_latents.shape == (B, C, TX) and x_lengths.shape == (B, TX)
    assert x_mask.shape == (B, 1, TX)

    mask = x_mask[:, 0, :]
    if not np.all((mask == 0.0) | (mask == 1.0)):
        return _np_reference(x_latents, x_lengths, x_mask, y_len, y_offset)

    ends = np.cumsum(x_lengths, axis=-1, dtype=np.float32)
    centers = (ends - np.float32(0.5) * x_lengths).astype(np.float32)
    centers_off = (centers - y_offset.astype(np.float32)[:, None]).astype(np.float32)
    centers_dev = np.where(mask > 0, centers_off, np.float32(FAR_CENTER))

    plan = _plan(centers_dev.astype(np.float64), mask)
    if plan is None:
        return _np_reference(x_latents, x_lengths, x_mask, y_len, y_offset)
    windows, strips, sneed, needy = plan

    use_fp32r = os.environ.get("ALIGNER_FP32", "0") != "1"
    nc = _build_program(windows, strips, needy, use_fp32r)

    # host-side input prep
    xa = np.empty((B, TX, C + 2), dtype=np.float32)
    xa[..., :C] = (x_latents * x_mask).transpose(0, 2, 1)
    xa[..., C] = mask
    xa[..., C + 1] = 0.0
    cen_lay = np.ascontiguousarray(
        centers_dev.reshape(B, NPO, 128).transpose(0, 2, 1))   # [B, pi, po]
    ar_np = np.ascontiguousarray(
        np.broadcast_to(np.arange(TY, dtype=np.float32), (128, TY)))

    in_maps = []
    for core in range(N_CORES):
        bs = slice(core * NB, (core + 1) * NB)
        im = {
            "xt": np.ascontiguousarray(xa[bs]),
            "cen": np.ascontiguousarray(cen_lay[bs]),
            "ar": ar_np,
        }
        for po in sorted(needy):
            ys, ye = strips[po]
            im[f"sh{po}"] = np.ascontiguousarray(
                np.broadcast_to(sneed[bs, None, ys:ye], (NB, 128, ye - ys)))
        in_maps.append(im)

    from concourse.bass_utils import run_bass_kernel_spmd

    prof_dir = os.environ.get("ALIGNER_PROF_DIR")
    hook = _make_ntff_hook() if prof_dir else None
    if hook is not None:
        os.makedirs(prof_dir, exist_ok=True)
        with hook(prof_dir, list(range(N_CORES))):
            res = run_bass_kernel_spmd(nc, in_maps, core_ids=list(range(N_CORES)))
    else:
        res = run_bass_kernel_spmd(nc, in_maps, core_ids=list(range(N_CORES)))

    full = np.concatenate([res.results[c]["out"] for c in range(N_CORES)],
                          axis=0)                     # [B, TY, C]
    return np.ascontiguousarray(full.transpose(0, 2, 1))


def _make_ntff_hook():
    try:
        from trn_agent_boot.trn_boot import _ntff_profile_via_ctypes
        return _ntff_profile_via_ctypes("/opt/axon/libaxon_pjrt.so")
    except Exception:
        return None


if __name__ == "__main__":
    import reference
    inputs = {k: np.asarray(v) for k, v in reference.setup_inputs().items()}
    got = kernel(**inputs)
    print("kernel output:", got.shape, got.dtype)


# revision 5
# speedup vs baseline: 1.1420x; 1.1420x over previous
"""Trainium2 Bass kernel for the Gaussian-alignment attention (Aligner) module.

Computes, for B=16, C=512, TX=1024, TY=2048:
    centers[b,t] = cumsum(x_lengths)[b,t] - 0.5*x_lengths[b,t]
    logits[b,t,y] = -(centers[b,t] - (y + y_offset[b]))^2 / 10 - 1e9*(1-mask[b,t])
    out[b,c,y] = sum_t x_latents[b,c,t] * softmax_t(logits)[b,t,y]

Strategy: data-parallel over batch across 8 NeuronCores (2 batches/core).
The alignment matrix is a narrow diagonal band (Gaussian with sigma ~2.2 in
position units), so exp() weights and the contraction are only evaluated on
data-dependent 128-wide t-blocks per 128-wide y-tile.  Per y-tile:
    E[t,y] = exp((-(c[t]-y)^2 + d2min[y]) / 10)   (ACT Square w/ per-partition
                                                   bias -> optional DVE shift
                                                   subtract -> ACT Exp)
    num    = E^T(y,t) . [x*mask]^T(t,c)           (PE matmul, t contracted)
    den    = E^T(y,t) . mask(t)                   (PE matmul, N=2 padded)
    out[y,c] = num * (1/den)                      (DVE reciprocal + t_scalar)
The d2min[y] shift is the exact softmax max-subtraction of the reference; it
is only materialized for t-blocks where some output frame is far from every
center (cumsum tails).  Masked tokens get their center pushed to +1e9 so
their weights are exactly 0.  The instruction stream is shared by all 8
cores (SPMD), so windows are unioned over all 16 batches.
"""

import os
import sys

sys.path.insert(0, "/opt/trn_rl_repo")

import numpy as np

B, C, TX, TY = 16, 512, 1024, 2048
GAMMA = 10.0
N_CORES = 8
NB = B // N_CORES          # batches per core
NPO = TX // 128            # t-blocks
NYT = TY // 128            # y-tiles
PAD_D2 = 1050.0            # keep terms until exp() underflows past subnormals
SHIFT_NEEDED_D2 = 600.0    # frames with d2min above this get the exact shift
FAR_CENTER = 1.0e9         # masked-token center placement


def _np_reference(x_latents, x_lengths, x_mask, y_len, y_offset):
    """Exact-semantics numpy fallback (used only for pathological inputs)."""
    out = np.empty((B, C, TY), dtype=np.float32)
    for b in range(B):
        ends = np.cumsum(x_lengths[b], dtype=np.float32)
        cen = (ends - np.float32(0.5) * x_lengths[b]).astype(np.float32)
        pos = (np.arange(TY, dtype=np.int64) + int(y_offset[b])).astype(np.float32)
        dist = cen[:, None] - pos[None, :]
        logits = -(dist * dist) / np.float32(GAMMA)
        logits = logits - np.float32(1e9) * (np.float32(1.0) - x_mask[b, 0][:, None])
        m = logits.max(axis=0, keepdims=True)
        w = np.exp(logits - m)
        w = w / w.sum(axis=0, keepdims=True)
        out[b] = x_latents[b].astype(np.float32) @ w
    return out


def _plan(centers_dev, mask):
    """Per-y-tile t-block windows, per-t-block y-strips and shift needs,
    unioned over all batches (the SPMD program is shared by all cores).

    centers_dev: [B, TX] float64, masked entries already at FAR_CENTER.
    Returns (windows, strips, sneed, needy) or None (-> numpy fallback).
      windows[yt] = (po_lo, po_hi) inclusive
      strips[po]  = (ys, ye) or None
      sneed       = [B, TY] float32 shift rows (thresholded d2min, the
                    reference's softmax max-subtraction where it matters)
      needy       = set of po whose strip applies the shift subtract
    """
    pos = np.arange(TY, dtype=np.float64)
    d2min = np.empty((B, TY), dtype=np.float32)
    for b in range(B):
        cm = centers_dev[b][centers_dev[b] < FAR_CENTER / 2]
        if cm.size == 0:
            return None
        cs = np.sort(cm)
        idx = np.searchsorted(cs, pos)
        lo = np.clip(idx - 1, 0, cs.size - 1)
        hi = np.clip(idx, 0, cs.size - 1)
        dmin = np.minimum(np.abs(cs[lo] - pos), np.abs(cs[hi] - pos))
        d2min[b] = (dmin * dmin).astype(np.float32)

    windows = []
    for yt in range(NYT):
        y0, y1 = 128 * yt, 128 * yt + 127
        r = float(np.sqrt(d2min[:, y0:y1 + 1].max() + PAD_D2))
        plo, phi = NPO, -1
        for b in range(B):
            t = np.nonzero((centers_dev[b] >= y0 - r) &
                           (centers_dev[b] <= y1 + r))[0]
            if t.size:
                plo = min(plo, int(t[0]) // 128)
                phi = max(phi, int(t[-1]) // 128)
        if phi < 0:
            return None
        windows.append((plo, phi))

    strips = [None] * NPO
    for po in range(NPO):
        yts = [yt for yt in range(NYT) if windows[yt][0] <= po <= windows[yt][1]]
        if yts:
            strips[po] = (128 * min(yts), 128 * max(yts) + 128)

    # The shift is the reference's softmax max-subtraction; it must be applied
    # to either all or none of a frame's terms, so threshold it to zero except
    # for deep-tail frames and close "needy" over whole y-tile windows.
    sneed = np.where(d2min > SHIFT_NEEDED_D2, d2min, 0.0).astype(np.float32)
    needy = set()
    for yt in range(NYT):
        y0 = 128 * yt
        if sneed[:, y0:y0 + 128].max() > 0:
            needy.update(range(windows[yt][0], windows[yt][1] + 1))
    return windows, strips, sneed, needy


def _build_program(windows, strips, needy, use_fp32r):
    import concourse.mybir as mybir
    import concourse.tile as tile
    from concourse import bacc

    F32 = mybir.dt.float32
    DT = mybir.dt.float32r if use_fp32r else F32
    ACTF = mybir.ActivationFunctionType

    nc = bacc.Bacc("TRN2", target_bir_lowering=False, debug=False,
                   num_devices=N_CORES)
    xt = nc.dram_tensor("xt", [NB, TX, C + 2], DT, kind="ExternalInput")
    cen = nc.dram_tensor("cen", [NB, 128, NPO], F32, kind="ExternalInput")
    sh = {po: nc.dram_tensor(f"sh{po}", [NB, 128, strips[po][1] - strips[po][0]],
                             F32, kind="ExternalInput")
          for po in sorted(needy)}
    out = nc.dram_tensor("out", [NB, TY, C], F32, kind="ExternalOutput")
    dbg = nc.dram_tensor("dbg", [128, 2], F32, kind="ExternalOutput")

    BF16 = mybir.dt.bfloat16
    with tile.TileContext(nc) as tc:
        with (
            tc.tile_pool(name="const", bufs=1) as constp,
            tc.tile_pool(name="xtp", bufs=2) as xtp,
            tc.tile_pool(name="cenp", bufs=2) as cenp,
            tc.tile_pool(name="sqp", bufs=2) as sqp,
            tc.tile_pool(name="shp", bufs=2) as shp,
            tc.tile_pool(name="ep", bufs=2) as epool,
            tc.tile_pool(name="pnum", bufs=1, space="PSUM") as pnum_pool,
            tc.tile_pool(name="pden", bufs=1, space="PSUM") as pden_pool,
            tc.tile_pool(name="wu", bufs=1, space="PSUM") as wu_pool,
            tc.tile_pool(name="wrk", bufs=4) as wrk,
            tc.tile_pool(name="outp", bufs=3) as outp,
        ):
            # PE warm-up: dense bf16 matmuls while the input DMAs stream in,
            # so the HAM clock gate is at 8/8 when the real work arrives.
            wu_sb = constp.tile([128, 512], BF16)
            nc.gpsimd.memset(wu_sb[:], 0.0)
            wu_ps = wu_pool.tile([128, 512], F32, tag="wu")
            for i in range(24):
                nc.tensor.matmul(wu_ps[:], wu_sb[:, 0:128], wu_sb[:],
                                 start=True, stop=True)
            wu_out = wrk.tile([128, 2], F32, tag="wuout")
            nc.vector.tensor_copy(wu_out[:], wu_ps[:, 0:2])
            nc.sync.dma_start(dbg[:], wu_out[:])

            ar_sb = constp.tile([128, TY], F32)
            nc.gpsimd.iota(ar_sb[:], pattern=[[1, TY]], base=0,
                           channel_multiplier=0,
                           allow_small_or_imprecise_dtypes=True)

            for slot in range(NB):
                xt_sb = xtp.tile([128, NPO, C + 2], DT, tag="xt")
                xt_r = xt[slot].rearrange("(po pi) c -> pi po c", pi=128)
                for po in range(NPO):
                    nc.sync.dma_start(xt_sb[:, po:po + 1, :], xt_r[:, po:po + 1, :])
                cen_sb = cenp.tile([128, NPO], F32, tag="cen")
                nc.sync.dma_start(cen_sb[:], cen[slot])

                e_strips = {}
                for po in range(NPO):
                    if strips[po] is None:
                        continue
                    ys, ye = strips[po]
                    sq = sqp.tile([128, ye - ys], F32, tag=f"sq{po}")
                    nc.scalar.activation(sq[:], ar_sb[:, ys:ye], ACTF.Square,
                                         bias=cen_sb[:, po:po + 1], scale=-1.0)
                    if po in needy:
                        sh_sb = shp.tile([128, ye - ys], F32, tag=f"sh{po}")
                        nc.sync.dma_start(sh_sb[:], sh[po][slot])
                        nc.vector.tensor_sub(sq[:], sq[:], sh_sb[:])
                    ep = epool.tile([128, ye - ys], DT, tag=f"e{po}")
                    nc.scalar.activation(ep[:], sq[:], ACTF.Exp,
                                         scale=-1.0 / GAMMA)
                    e_strips[po] = ep

                for yt in range(NYT):
                    y0 = 128 * yt
                    plo, phi = windows[yt]
                    pnum = pnum_pool.tile([128, C], mybir.dt.float32,
                                          tag=f"pn{yt % 3}")
                    pden = pden_pool.tile([128, 2], mybir.dt.float32,
                                          tag=f"pd{yt % 4}")
                    for j, po in enumerate(range(plo, phi + 1)):
                        ys, _ = strips[po]
                        w_ap = e_strips[po][:, y0 - ys:y0 - ys + 128]
                        nc.tensor.matmul(pnum[:], w_ap, xt_sb[:, po, 0:C],
                                         start=(j == 0), stop=(po == phi))
                        nc.tensor.matmul(pden[:], w_ap, xt_sb[:, po, C:C + 2],
                                         start=(j == 0), stop=(po == phi))
                    recip = wrk.tile([128, 1], F32, tag="recip")
                    nc.vector.reciprocal(recip[:], pden[:, 0:1])
                    ob = outp.tile([128, C], F32, tag=f"ob{yt % 3}")
                    nc.vector.tensor_scalar_mul(ob[:], pnum[:], recip[:])
                    nc.sync.dma_start(out[slot, y0:y0 + 128, :], ob[:])

    nc.finalize()
    return nc


def kernel(x_latents, x_lengths, x_mask, y_len, y_offset):
    x_latents = np.ascontiguousarray(np.asarray(x_latents, dtype=np.float32))
    x_lengths = np.ascontiguousarray(np.asarray(x_lengths, dtype=np.float32))
    x_mask = np.ascontiguousarray(np.asarray(x_mask, dtype=np.float32))
    y_len = np.asarray(y_len)
    y_offset = np.asarray(y_offset)
    assert x_latents.shape == (B, C, TX) and x_lengths.shape == (B, TX)
    assert x_mask.shape == (B, 1, TX)

    mask = x_mask[:, 0, :]
    if not np.all((mask == 0.0) | (mask == 1.0)):
        return _np_reference(x_latents, x_lengths, x_mask, y_len, y_offset)

    ends = np.cumsum(x_lengths, axis=-1, dtype=np.float32)
    centers = (ends - np.float32(0.5) * x_lengths).astype(np.float32)
    centers_off = (centers - y_offset.astype(np.float32)[:, None]).astype(np.float32)
    centers_dev = np.where(mask > 0, centers_off, np.float32(FAR_CENTER))

    plan = _plan(centers_dev.astype(np.float64), mask)
    if plan is None:
        return _np_reference(x_latents, x_lengths, x_mask, y_len, y_offset)
    windows, strips, sneed, needy = plan

    use_fp32r = os.environ.get("ALIGNER_FP32", "0") != "1"
    nc = _build_program(windows, strips, needy, use_fp32r)

    # host-side input prep
    xa = np.empty((B, TX, C + 2), dtype=np.float32)
    xa[..., :C] = (x_latents * x_mask).transpose(0, 2, 1)
    xa[..., C] = mask
    xa[..., C + 1] = 0.0
    cen_lay = np.ascontiguousarray(
        centers_dev.reshape(B, NPO, 128).transpose(0, 2, 1))   # [B, pi, po]
    ar_np = np.ascontiguousarray(
        np.broadcast_to(np.arange(TY, dtype=np.float32), (128, TY)))

    in_maps = []
    for core in range(N_CORES):
        bs = slice(core * NB, (core + 1) * NB)
        im = {
            "xt": np.ascontiguousarray(xa[bs]),
            "cen": np.ascontiguousarray(cen_lay[bs]),
            "ar": ar_np,
        }
        for po in sorted(needy):
            ys, ye = strips[po]
            im[f"sh{po}"] = np.ascontiguousarray(
                np.broadcast_to(sneed[bs, None, ys:ye], (NB, 128, ye - ys)))
        in_maps.append(im)

    from concourse.bass_utils import run_bass_kernel_spmd

    prof_dir = os.environ.get("ALIGNER_PROF_DIR")
    hook = _make_ntff_hook() if prof_dir else None
    if hook is not None:
        os.makedirs(prof_dir, exist_ok=True)
        with hook(prof_dir, list(range(N_CORES))):
            res = run_bass_kernel_spmd(nc, in_maps, core_ids=list(range(N_CORES)))
    else:
        res = run_bass_kernel_spmd(nc, in_maps, core_ids=list(range(N_CORES)))

    full = np.concatenate([res.results[c]["out"] for c in range(N_CORES)],
                          axis=0)                     # [B, TY, C]
    return np.ascontiguousarray(full.transpose(0, 2, 1))


def _make_ntff_hook():
    try:
        from trn_agent_boot.trn_boot import _ntff_profile_via_ctypes
        return _ntff_profile_via_ctypes("/opt/axon/libaxon_pjrt.so")
    except Exception:
        return None


if __name__ == "__main__":
    import reference
    inputs = {k: np.asarray(v) for k, v in reference.setup_inputs().items()}
    got = kernel(**inputs)
    print("kernel output:", got.shape, got.dtype)


# revision 6
# speedup vs baseline: 1.3196x; 1.1555x over previous
"""Trainium2 Bass kernel for the Gaussian-alignment attention (Aligner) module.

Computes, for B=16, C=512, TX=1024, TY=2048:
    centers[b,t] = cumsum(x_lengths)[b,t] - 0.5*x_lengths[b,t]
    logits[b,t,y] = -(centers[b,t] - (y + y_offset[b]))^2 / 10 - 1e9*(1-mask[b,t])
    out[b,c,y] = sum_t x_latents[b,c,t] * softmax_t(logits)[b,t,y]

Strategy: data-parallel over batch across 8 NeuronCores (2 batches/core).
The alignment matrix is a narrow diagonal band (Gaussian with sigma ~2.2 in
position units), so exp() weights and the contraction are only evaluated on
data-dependent 128-wide t-blocks per 128-wide y-tile.  Per y-tile:
    E[t,y] = exp((-(c[t]-y)^2 + d2min[y]) / 10)   (ACT Square w/ per-partition
                                                   bias -> optional DVE shift
                                                   subtract -> ACT Exp)
    num    = E^T(y,t) . [x*mask]^T(t,c)           (PE matmul, t contracted)
    den    = E^T(y,t) . mask(t)                   (PE matmul, N=2 padded)
    out[y,c] = num * (1/den)                      (DVE reciprocal + t_scalar)
The d2min[y] shift is the exact softmax max-subtraction of the reference; it
is only materialized for t-blocks where some output frame is far from every
center (cumsum tails).  Masked tokens get their center pushed to +1e9 so
their weights are exactly 0.  The instruction stream is shared by all 8
cores (SPMD), so windows are unioned over all 16 batches.
"""

import os
import sys

sys.path.insert(0, "/opt/trn_rl_repo")

import numpy as np

B, C, TX, TY = 16, 512, 1024, 2048
GAMMA = 10.0
N_CORES = 8
NB = B // N_CORES          # batches per core
NPO = TX // 128            # t-blocks
NYT = TY // 128            # y-tiles
PAD_D2 = 1050.0            # keep terms until exp() underflows past subnormals
SHIFT_NEEDED_D2 = 600.0    # frames with d2min above this get the exact shift
FAR_CENTER = 1.0e9         # masked-token center placement


def _np_reference(x_latents, x_lengths, x_mask, y_len, y_offset):
    """Exact-semantics numpy fallback (used only for pathological inputs)."""
    out = np.empty((B, C, TY), dtype=np.float32)
    for b in range(B):
        ends = np.cumsum(x_lengths[b], dtype=np.float32)
        cen = (ends - np.float32(0.5) * x_lengths[b]).astype(np.float32)
        pos = (np.arange(TY, dtype=np.int64) + int(y_offset[b])).astype(np.float32)
        dist = cen[:, None] - pos[None, :]
        logits = -(dist * dist) / np.float32(GAMMA)
        logits = logits - np.float32(1e9) * (np.float32(1.0) - x_mask[b, 0][:, None])
        m = logits.max(axis=0, keepdims=True)
        w = np.exp(logits - m)
        w = w / w.sum(axis=0, keepdims=True)
        out[b] = x_latents[b].astype(np.float32) @ w
    return out


def _plan(centers_dev, mask):
    """Per-y-tile t-block windows, per-t-block y-strips and shift needs,
    unioned over all batches (the SPMD program is shared by all cores).

    centers_dev: [B, TX] float64, masked entries already at FAR_CENTER.
    Returns (windows, strips, sneed, needy) or None (-> numpy fallback).
      windows[yt] = (po_lo, po_hi) inclusive
      strips[po]  = (ys, ye) or None
      sneed       = [B, TY] float32 shift rows (thresholded d2min, the
                    reference's softmax max-subtraction where it matters)
      needy       = set of po whose strip applies the shift subtract
    """
    pos = np.arange(TY, dtype=np.float64)
    d2min = np.empty((B, TY), dtype=np.float32)
    for b in range(B):
        cm = centers_dev[b][centers_dev[b] < FAR_CENTER / 2]
        if cm.size == 0:
            return None
        cs = np.sort(cm)
        idx = np.searchsorted(cs, pos)
        lo = np.clip(idx - 1, 0, cs.size - 1)
        hi = np.clip(idx, 0, cs.size - 1)
        dmin = np.minimum(np.abs(cs[lo] - pos), np.abs(cs[hi] - pos))
        d2min[b] = (dmin * dmin).astype(np.float32)

    windows = []
    for yt in range(NYT):
        y0, y1 = 128 * yt, 128 * yt + 127
        r = float(np.sqrt(d2min[:, y0:y1 + 1].max() + PAD_D2))
        plo, phi = NPO, -1
        for b in range(B):
            t = np.nonzero((centers_dev[b] >= y0 - r) &
                           (centers_dev[b] <= y1 + r))[0]
            if t.size:
                plo = min(plo, int(t[0]) // 128)
                phi = max(phi, int(t[-1]) // 128)
        if phi < 0:
            return None
        windows.append((plo, phi))

    strips = [None] * NPO
    for po in range(NPO):
        yts = [yt for yt in range(NYT) if windows[yt][0] <= po <= windows[yt][1]]
        if yts:
            strips[po] = (128 * min(yts), 128 * max(yts) + 128)

    # The shift is the reference's softmax max-subtraction; it must be applied
    # to either all or none of a frame's terms, so threshold it to zero except
    # for deep-tail frames and close "needy" over whole y-tile windows.
    sneed = np.where(d2min > SHIFT_NEEDED_D2, d2min, 0.0).astype(np.float32)
    needy = set()
    for yt in range(NYT):
        y0 = 128 * yt
        if sneed[:, y0:y0 + 128].max() > 0:
            needy.update(range(windows[yt][0], windows[yt][1] + 1))
    return windows, strips, sneed, needy


def _build_program(windows, strips, needy, use_fp32r):
    import concourse.mybir as mybir
    import concourse.tile as tile
    from concourse import bacc

    F32 = mybir.dt.float32
    DT = mybir.dt.float32r if use_fp32r else F32
    ACTF = mybir.ActivationFunctionType

    nc = bacc.Bacc("TRN2", target_bir_lowering=False, debug=False,
                   num_devices=N_CORES)
    xt = nc.dram_tensor("xt", [NB, TX, C + 2], DT, kind="ExternalInput")
    cen = nc.dram_tensor("cen", [NB, 128, NPO], F32, kind="ExternalInput")
    sh = {po: nc.dram_tensor(f"sh{po}", [NB, 128, strips[po][1] - strips[po][0]],
                             F32, kind="ExternalInput")
          for po in sorted(needy)}
    out = nc.dram_tensor("out", [NB, TY, C], F32, kind="ExternalOutput")
    dbg = nc.dram_tensor("dbg", [128, 2], F32, kind="ExternalOutput")

    BF16 = mybir.dt.bfloat16
    with tile.TileContext(nc) as tc:
        with (
            tc.tile_pool(name="const", bufs=1) as constp,
            tc.tile_pool(name="xtp", bufs=2) as xtp,
            tc.tile_pool(name="cenp", bufs=2) as cenp,
            tc.tile_pool(name="sqp", bufs=2) as sqp,
            tc.tile_pool(name="shp", bufs=2) as shp,
            tc.tile_pool(name="ep", bufs=2) as epool,
            tc.tile_pool(name="pnum", bufs=1, space="PSUM") as pnum_pool,
            tc.tile_pool(name="pden", bufs=1, space="PSUM") as pden_pool,
            tc.tile_pool(name="wu", bufs=1, space="PSUM") as wu_pool,
            tc.tile_pool(name="wrk", bufs=4) as wrk,
            tc.tile_pool(name="outp", bufs=3) as outp,
        ):
            # PE warm-up: dense bf16 matmuls while the input DMAs stream in,
            # so the HAM clock gate is at 8/8 when the real work arrives.
            wu_sb = constp.tile([128, 512], BF16)
            nc.gpsimd.memset(wu_sb[:], 0.0)
            wu_ps = wu_pool.tile([128, 512], F32, tag="wu")
            for i in range(24):
                nc.tensor.matmul(wu_ps[:], wu_sb[:, 0:128], wu_sb[:],
                                 start=True, stop=True)
            wu_out = wrk.tile([128, 2], F32, tag="wuout")
            nc.vector.tensor_copy(wu_out[:], wu_ps[:, 0:2])
            nc.sync.dma_start(dbg[:], wu_out[:])

            ar_sb = constp.tile([128, TY], F32)
            nc.gpsimd.iota(ar_sb[:], pattern=[[1, TY]], base=0,
                           channel_multiplier=0,
                           allow_small_or_imprecise_dtypes=True)

            for slot in range(NB):
                # small DMAs first: they gate the ACT strip generation
                cen_sb = cenp.tile([128, NPO], F32, tag="cen")
                nc.sync.dma_start(cen_sb[:], cen[slot])
                sh_sbs = {}
                for po in sorted(needy):
                    ys, ye = strips[po]
                    sh_sb = shp.tile([128, ye - ys], F32, tag=f"sh{po}")
                    nc.sync.dma_start(sh_sb[:], sh[po][slot])
                    sh_sbs[po] = sh_sb
                xt_sb = xtp.tile([128, NPO, C + 2], DT, tag="xt")
                xt_r = xt[slot].rearrange("(po pi) c -> pi po c", pi=128)
                for po in range(NPO):
                    nc.sync.dma_start(xt_sb[:, po:po + 1, :], xt_r[:, po:po + 1, :])

                e_strips = {}
                for po in range(NPO):
                    if strips[po] is None:
                        continue
                    ys, ye = strips[po]
                    sq = sqp.tile([128, ye - ys], F32, tag=f"sq{po}")
                    nc.scalar.activation(sq[:], ar_sb[:, ys:ye], ACTF.Square,
                                         bias=cen_sb[:, po:po + 1], scale=-1.0)
                    if po in needy:
                        nc.vector.tensor_sub(sq[:], sq[:], sh_sbs[po][:])
                    ep = epool.tile([128, ye - ys], DT, tag=f"e{po}")
                    nc.scalar.activation(ep[:], sq[:], ACTF.Exp,
                                         scale=-1.0 / GAMMA)
                    e_strips[po] = ep

                for yt in range(NYT):
                    y0 = 128 * yt
                    plo, phi = windows[yt]
                    pnum = pnum_pool.tile([128, C], mybir.dt.float32,
                                          tag=f"pn{yt % 3}")
                    pden = pden_pool.tile([128, 2], mybir.dt.float32,
                                          tag=f"pd{yt % 4}")
                    for j, po in enumerate(range(plo, phi + 1)):
                        ys, _ = strips[po]
                        w_ap = e_strips[po][:, y0 - ys:y0 - ys + 128]
                        nc.tensor.matmul(pnum[:], w_ap, xt_sb[:, po, 0:C],
                                         start=(j == 0), stop=(po == phi))
                        nc.tensor.matmul(pden[:], w_ap, xt_sb[:, po, C:C + 2],
                                         start=(j == 0), stop=(po == phi))
                    recip = wrk.tile([128, 1], F32, tag="recip")
                    nc.vector.reciprocal(recip[:], pden[:, 0:1])
                    ob = outp.tile([128, C], F32, tag=f"ob{yt % 3}")
                    nc.vector.tensor_scalar_mul(ob[:], pnum[:], recip[:])
                    nc.sync.dma_start(out[slot, y0:y0 + 128, :], ob[:])

    nc.finalize()
    return nc


def kernel(x_latents, x_lengths, x_mask, y_len, y_offset):
    x_latents = np.ascontiguousarray(np.asarray(x_latents, dtype=np.float32))
    x_lengths = np.ascontiguousarray(np.asarray(x_lengths, dtype=np.float32))
    x_mask = np.ascontiguousarray(np.asarray(x_mask, dtype=np.float32))
    y_len = np.asarray(y_len)
    y_offset = np.asarray(y_offset)
    assert x_latents.shape == (B, C, TX) and x_lengths.shape == (B, TX)
    assert x_mask.shape == (B, 1, TX)

    mask = x_mask[:, 0, :]
    if not np.all((mask == 0.0) | (mask == 1.0)):
        return _np_reference(x_latents, x_lengths, x_mask, y_len, y_offset)

    ends = np.cumsum(x_lengths, axis=-1, dtype=np.float32)
    centers = (ends - np.float32(0.5) * x_lengths).astype(np.float32)
    centers_off = (centers - y_offset.astype(np.float32)[:, None]).astype(np.float32)
    centers_dev = np.where(mask > 0, centers_off, np.float32(FAR_CENTER))

    plan = _plan(centers_dev.astype(np.float64), mask)
    if plan is None:
        return _np_reference(x_latents, x_lengths, x_mask, y_len, y_offset)
    windows, strips, sneed, needy = plan

    use_fp32r = os.environ.get("ALIGNER_FP32", "0") != "1"
    nc = _build_program(windows, strips, needy, use_fp32r)

    # host-side input prep
    xa = np.empty((B, TX, C + 2), dtype=np.float32)
    xa[..., :C] = (x_latents * x_mask).transpose(0, 2, 1)
    xa[..., C] = mask
    xa[..., C + 1] = 0.0
    cen_lay = np.ascontiguousarray(
        centers_dev.reshape(B, NPO, 128).transpose(0, 2, 1))   # [B, pi, po]
    ar_np = np.ascontiguousarray(
        np.broadcast_to(np.arange(TY, dtype=np.float32), (128, TY)))

    in_maps = []
    for core in range(N_CORES):
        bs = slice(core * NB, (core + 1) * NB)
        im = {
            "xt": np.ascontiguousarray(xa[bs]),
            "cen": np.ascontiguousarray(cen_lay[bs]),
            "ar": ar_np,
        }
        for po in sorted(needy):
            ys, ye = strips[po]
            im[f"sh{po}"] = np.ascontiguousarray(
                np.broadcast_to(sneed[bs, None, ys:ye], (NB, 128, ye - ys)))
        in_maps.append(im)

    from concourse.bass_utils import run_bass_kernel_spmd

    prof_dir = os.environ.get("ALIGNER_PROF_DIR")
    hook = _make_ntff_hook() if prof_dir else None
    if hook is not None:
        os.makedirs(prof_dir, exist_ok=True)
        with hook(prof_dir, list(range(N_CORES))):
            res = run_bass_kernel_spmd(nc, in_maps, core_ids=list(range(N_CORES)))
    else:
        res = run_bass_kernel_spmd(nc, in_maps, core_ids=list(range(N_CORES)))

    full = np.concatenate([res.results[c]["out"] for c in range(N_CORES)],
                          axis=0)                     # [B, TY, C]
    return np.ascontiguousarray(full.transpose(0, 2, 1))


def _make_ntff_hook():
    try:
        from trn_agent_boot.trn_boot import _ntff_profile_via_ctypes
        return _ntff_profile_via_ctypes("/opt/axon/libaxon_pjrt.so")
    except Exception:
        return None


if __name__ == "__main__":
    import reference
    inputs = {k: np.asarray(v) for k, v in reference.setup_inputs().items()}
    got = kernel(**inputs)
    print("kernel output:", got.shape, got.dtype)


# revision 8
# speedup vs baseline: 1.4502x; 1.0990x over previous
"""Trainium2 Bass kernel for the Gaussian-alignment attention (Aligner) module.

Computes, for B=16, C=512, TX=1024, TY=2048:
    centers[b,t] = cumsum(x_lengths)[b,t] - 0.5*x_lengths[b,t]
    logits[b,t,y] = -(centers[b,t] - (y + y_offset[b]))^2 / 10 - 1e9*(1-mask[b,t])
    out[b,c,y] = sum_t x_latents[b,c,t] * softmax_t(logits)[b,t,y]

Strategy: data-parallel over batch across 8 NeuronCores (2 batches/core).
The alignment matrix is a narrow diagonal band (Gaussian with sigma ~2.2 in
position units), so exp() weights and the contraction are only evaluated on
data-dependent 128-wide t-blocks per 128-wide y-tile.  Per y-tile:
    E[t,y] = exp((-(c[t]-y)^2 + d2min[y]) / 10)   (ACT Square w/ per-partition
                                                   bias -> optional DVE shift
                                                   subtract -> ACT Exp)
    num    = E^T(y,t) . [x*mask]^T(t,c)           (PE matmul, t contracted)
    den    = E^T(y,t) . mask(t)                   (PE matmul, N=2 padded)
    out[y,c] = num * (1/den)                      (DVE reciprocal + t_scalar)
The d2min[y] shift is the exact softmax max-subtraction of the reference; it
is only materialized for t-blocks where some output frame is far from every
center (cumsum tails).  Masked tokens get their center pushed to +1e9 so
their weights are exactly 0.  The instruction stream is shared by all 8
cores (SPMD), so windows are unioned over all 16 batches.
"""

import os
import sys

sys.path.insert(0, "/opt/trn_rl_repo")

import numpy as np

B, C, TX, TY = 16, 512, 1024, 2048
GAMMA = 10.0
N_CORES = 8
NB = B // N_CORES          # batches per core
NPO = TX // 128            # t-blocks
NYT = TY // 128            # y-tiles
PAD_D2 = 1050.0            # keep terms until exp() underflows past subnormals
SHIFT_NEEDED_D2 = 600.0    # frames with d2min above this get the exact shift
FAR_CENTER = 1.0e9         # masked-token center placement


def _np_reference(x_latents, x_lengths, x_mask, y_len, y_offset):
    """Exact-semantics numpy fallback (used only for pathological inputs)."""
    out = np.empty((B, C, TY), dtype=np.float32)
    for b in range(B):
        ends = np.cumsum(x_lengths[b], dtype=np.float32)
        cen = (ends - np.float32(0.5) * x_lengths[b]).astype(np.float32)
        pos = (np.arange(TY, dtype=np.int64) + int(y_offset[b])).astype(np.float32)
        dist = cen[:, None] - pos[None, :]
        logits = -(dist * dist) / np.float32(GAMMA)
        logits = logits - np.float32(1e9) * (np.float32(1.0) - x_mask[b, 0][:, None])
        m = logits.max(axis=0, keepdims=True)
        w = np.exp(logits - m)
        w = w / w.sum(axis=0, keepdims=True)
        out[b] = x_latents[b].astype(np.float32) @ w
    return out


def _plan(centers_dev, mask):
    """Per-y-tile t-block windows, per-t-block y-strips and shift needs,
    unioned over all batches (the SPMD program is shared by all cores).

    centers_dev: [B, TX] float64, masked entries already at FAR_CENTER.
    Returns (windows, strips, sneed, needy) or None (-> numpy fallback).
      windows[yt] = (po_lo, po_hi) inclusive
      strips[po]  = (ys, ye) or None
      sneed       = [B, TY] float32 shift rows (thresholded d2min, the
                    reference's softmax max-subtraction where it matters)
      needy       = set of po whose strip applies the shift subtract
    """
    pos = np.arange(TY, dtype=np.float64)
    d2min = np.empty((B, TY), dtype=np.float32)
    for b in range(B):
        cm = centers_dev[b][centers_dev[b] < FAR_CENTER / 2]
        if cm.size == 0:
            return None
        cs = np.sort(cm)
        idx = np.searchsorted(cs, pos)
        lo = np.clip(idx - 1, 0, cs.size - 1)
        hi = np.clip(idx, 0, cs.size - 1)
        dmin = np.minimum(np.abs(cs[lo] - pos), np.abs(cs[hi] - pos))
        d2min[b] = (dmin * dmin).astype(np.float32)

    windows = []
    for yt in range(NYT):
        y0, y1 = 128 * yt, 128 * yt + 127
        r = float(np.sqrt(d2min[:, y0:y1 + 1].max() + PAD_D2))
        plo, phi = NPO, -1
        for b in range(B):
            t = np.nonzero((centers_dev[b] >= y0 - r) &
                           (centers_dev[b] <= y1 + r))[0]
            if t.size:
                plo = min(plo, int(t[0]) // 128)
                phi = max(phi, int(t[-1]) // 128)
        if phi < 0:
            return None
        windows.append((plo, phi))

    strips = [None] * NPO
    for po in range(NPO):
        yts = [yt for yt in range(NYT) if windows[yt][0] <= po <= windows[yt][1]]
        if yts:
            strips[po] = (128 * min(yts), 128 * max(yts) + 128)

    # The shift is the reference's softmax max-subtraction; it must be applied
    # to either all or none of a frame's terms, so threshold it to zero except
    # for deep-tail frames and close "needy" over whole y-tile windows.
    sneed = np.where(d2min > SHIFT_NEEDED_D2, d2min, 0.0).astype(np.float32)
    needy = set()
    for yt in range(NYT):
        y0 = 128 * yt
        if sneed[:, y0:y0 + 128].max() > 0:
            needy.update(range(windows[yt][0], windows[yt][1] + 1))
    return windows, strips, sneed, needy


def _build_program(windows, strips, needy, use_fp32r):
    import concourse.mybir as mybir
    import concourse.tile as tile
    from concourse import bacc

    F32 = mybir.dt.float32
    DT = mybir.dt.float32r if use_fp32r else F32
    ACTF = mybir.ActivationFunctionType

    nc = bacc.Bacc("TRN2", target_bir_lowering=False, debug=False,
                   num_devices=N_CORES)
    xt = nc.dram_tensor("xt", [NB, TX, C + 2], DT, kind="ExternalInput")
    cen = nc.dram_tensor("cen", [NB, 128, NPO], F32, kind="ExternalInput")
    sh = {po: nc.dram_tensor(f"sh{po}", [NB, 128, strips[po][1] - strips[po][0]],
                             F32, kind="ExternalInput")
          for po in sorted(needy)}
    out = nc.dram_tensor("out", [NB, TY, C], F32, kind="ExternalOutput")
    dbg = nc.dram_tensor("dbg", [128, 2], F32, kind="ExternalOutput")

    BF16 = mybir.dt.bfloat16
    with tile.TileContext(nc) as tc:
        with (
            tc.tile_pool(name="const", bufs=1) as constp,
            tc.tile_pool(name="xtp", bufs=1) as xtp,
            tc.tile_pool(name="cenp", bufs=1) as cenp,
            tc.tile_pool(name="sqp", bufs=2) as sqp,
            tc.tile_pool(name="shp", bufs=1) as shp,
            tc.tile_pool(name="ep", bufs=2) as epool,
            tc.tile_pool(name="pnum", bufs=1, space="PSUM") as pnum_pool,
            tc.tile_pool(name="pden", bufs=1, space="PSUM") as pden_pool,
            tc.tile_pool(name="wu", bufs=1, space="PSUM") as wu_pool,
            tc.tile_pool(name="wrk", bufs=4) as wrk,
            tc.tile_pool(name="outp", bufs=3) as outp,
        ):
            # PE warm-up: dense bf16 matmuls while the input DMAs stream in,
            # so the HAM clock gate is at 8/8 when the real work arrives.
            wu_sb = constp.tile([128, 512], BF16)
            nc.gpsimd.memset(wu_sb[:], 0.0)
            wu_ps = wu_pool.tile([128, 512], F32, tag="wu")
            for i in range(24):
                nc.tensor.matmul(wu_ps[:], wu_sb[:, 0:128], wu_sb[:],
                                 start=True, stop=True)
            wu_out = wrk.tile([128, 2], F32, tag="wuout")
            nc.vector.tensor_copy(wu_out[:], wu_ps[:, 0:2])
            nc.sync.dma_start(dbg[:], wu_out[:])

            ar_sb = constp.tile([128, TY], F32)
            nc.gpsimd.iota(ar_sb[:], pattern=[[1, TY]], base=0,
                           channel_multiplier=0,
                           allow_small_or_imprecise_dtypes=True)

            # prefetch all input DMAs for both slots up front (SP issues them
            # in program order; the small ones gate ACT strip generation)
            cen_sbs, sh_sbss, xt_sbs = [], [], []
            for slot in range(NB):
                cen_sb = cenp.tile([128, NPO], F32, tag=f"cen{slot}")
                nc.sync.dma_start(cen_sb[:], cen[slot])
                cen_sbs.append(cen_sb)
                sh_sbs = {}
                for po in sorted(needy):
                    ys, ye = strips[po]
                    sh_sb = shp.tile([128, ye - ys], F32, tag=f"sh{po}_{slot}")
                    nc.sync.dma_start(sh_sb[:], sh[po][slot])
                    sh_sbs[po] = sh_sb
                sh_sbss.append(sh_sbs)
            for slot in range(NB):
                xt_sb = xtp.tile([128, NPO, C + 2], DT, tag=f"xt{slot}")
                xt_r = xt[slot].rearrange("(po pi) c -> pi po c", pi=128)
                for po in range(NPO):
                    nc.sync.dma_start(xt_sb[:, po:po + 1, :], xt_r[:, po:po + 1, :])
                xt_sbs.append(xt_sb)

            for slot in range(NB):
                cen_sb = cen_sbs[slot]
                sh_sbs = sh_sbss[slot]
                xt_sb = xt_sbs[slot]
                e_strips = {}
                for po in range(NPO):
                    if strips[po] is None:
                        continue
                    ys, ye = strips[po]
                    sq = sqp.tile([128, ye - ys], F32, tag=f"sq{po}")
                    nc.scalar.activation(sq[:], ar_sb[:, ys:ye], ACTF.Square,
                                         bias=cen_sb[:, po:po + 1], scale=-1.0)
                    if po in needy:
                        nc.vector.tensor_sub(sq[:], sq[:], sh_sbs[po][:])
                    ep = epool.tile([128, ye - ys], DT, tag=f"e{po}")
                    nc.scalar.activation(ep[:], sq[:], ACTF.Exp,
                                         scale=-1.0 / GAMMA)
                    e_strips[po] = ep

                for yt in range(NYT):
                    y0 = 128 * yt
                    plo, phi = windows[yt]
                    pnum = pnum_pool.tile([128, C], mybir.dt.float32,
                                          tag=f"pn{yt % 4}")
                    pden = pden_pool.tile([128, 2], mybir.dt.float32,
                                          tag=f"pd{yt % 2}")
                    for j, po in enumerate(range(plo, phi + 1)):
                        ys, _ = strips[po]
                        w_ap = e_strips[po][:, y0 - ys:y0 - ys + 128]
                        nc.tensor.matmul(pnum[:], w_ap, xt_sb[:, po, 0:C],
                                         start=(j == 0), stop=(po == phi))
                        nc.tensor.matmul(pden[:], w_ap, xt_sb[:, po, C:C + 2],
                                         start=(j == 0), stop=(po == phi))
                    recip = wrk.tile([128, 1], F32, tag="recip")
                    nc.vector.reciprocal(recip[:], pden[:, 0:1])
                    ob = outp.tile([128, C], F32, tag=f"ob{yt % 3}_{slot}")
                    nc.vector.tensor_scalar_mul(ob[:], pnum[:], recip[:])
                    nc.sync.dma_start(out[slot, y0:y0 + 128, :], ob[:])

    nc.finalize()
    return nc


def kernel(x_latents, x_lengths, x_mask, y_len, y_offset):
    x_latents = np.ascontiguousarray(np.asarray(x_latents, dtype=np.float32))
    x_lengths = np.ascontiguousarray(np.asarray(x_lengths, dtype=np.float32))
    x_mask = np.ascontiguousarray(np.asarray(x_mask, dtype=np.float32))
    y_len = np.asarray(y_len)
    y_offset = np.asarray(y_offset)
    assert x_latents.shape == (B, C, TX) and x_lengths.shape == (B, TX)
    assert x_mask.shape == (B, 1, TX)

    mask = x_mask[:, 0, :]
    if not np.all((mask == 0.0) | (mask == 1.0)):
        return _np_reference(x_latents, x_lengths, x_mask, y_len, y_offset)

    ends = np.cumsum(x_lengths, axis=-1, dtype=np.float32)
    centers = (ends - np.float32(0.5) * x_lengths).astype(np.float32)
    centers_off = (centers - y_offset.astype(np.float32)[:, None]).astype(np.float32)
    centers_dev = np.where(mask > 0, centers_off, np.float32(FAR_CENTER))

    plan = _plan(centers_dev.astype(np.float64), mask)
    if plan is None:
        return _np_reference(x_latents, x_lengths, x_mask, y_len, y_offset)
    windows, strips, sneed, needy = plan

    use_fp32r = os.environ.get("ALIGNER_FP32", "0") != "1"
    nc = _build_program(windows, strips, needy, use_fp32r)

    # host-side input prep
    xa = np.empty((B, TX, C + 2), dtype=np.float32)
    xa[..., :C] = (x_latents * x_mask).transpose(0, 2, 1)
    xa[..., C] = mask
    xa[..., C + 1] = 0.0
    cen_lay = np.ascontiguousarray(
        centers_dev.reshape(B, NPO, 128).transpose(0, 2, 1))   # [B, pi, po]
    ar_np = np.ascontiguousarray(
        np.broadcast_to(np.arange(TY, dtype=np.float32), (128, TY)))

    in_maps = []
    for core in range(N_CORES):
        bs = slice(core * NB, (core + 1) * NB)
        im = {
            "xt": np.ascontiguousarray(xa[bs]),
            "cen": np.ascontiguousarray(cen_lay[bs]),
            "ar": ar_np,
        }
        for po in sorted(needy):
            ys, ye = strips[po]
            im[f"sh{po}"] = np.ascontiguousarray(
                np.broadcast_to(sneed[bs, None, ys:ye], (NB, 128, ye - ys)))
        in_maps.append(im)

    from concourse.bass_utils import run_bass_kernel_spmd

    prof_dir = os.environ.get("ALIGNER_PROF_DIR")
    hook = _make_ntff_hook() if prof_dir else None
    if hook is not None:
        os.makedirs(prof_dir, exist_ok=True)
        with hook(prof_dir, list(range(N_CORES))):
            res = run_bass_kernel_spmd(nc, in_maps, core_ids=list(range(N_CORES)))
    else:
        res = run_bass_kernel_spmd(nc, in_maps, core_ids=list(range(N_CORES)))

    full = np.concatenate([res.results[c]["out"] for c in range(N_CORES)],
                          axis=0)                     # [B, TY, C]
    return np.ascontiguousarray(full.transpose(0, 2, 1))


def _make_ntff_hook():
    try:
        from trn_agent_boot.trn_boot import _ntff_profile_via_ctypes
        return _ntff_profile_via_ctypes("/opt/axon/libaxon_pjrt.so")
    except Exception:
        return None


if __name__ == "__main__":
    import reference
    inputs = {k: np.asarray(v) for k, v in reference.setup_inputs().items()}
    got = kernel(**inputs)
    print("kernel output:", got.shape, got.dtype)
